# revision 1
# baseline (speedup 1.0000x reference)
"""Trainium2 Bass kernel for nn_BiomechanicsLoss (masked quadratic-form loss).

Math (per point): et = [u0, v1, w2, .5(u1+v0), .5(u2+w0), .5(w1+v2)],
q = et^T C et with C = inv(compliance) cast to f32.  Loss =
sqrt(sum_masked(q^2)) / count_masked, mask = gt_sdf < 1e-8.

Because q = et^T C et == et^T sym(C) et and C is block-diagonal
(3x3 normal block + diagonal shear block), with Fm = diag(1,1,1,.5,.5,.5):
  q = w11*s1^2 + w22*s2^2 + w33*s3^2 + w12*s1*s2 + w13*s1*s3 + w23*s2*s3
      + d*(s4^2 + s5^2 + s6^2)
where s1..s3 = u0, v1, w2 ; s4 = u1+v0 ; s5 = u2+w0 ; s6 = w1+v2 and the
weights come from M = Fm*sym(C)*Fm (all positive for these constants).

Sharding: pure data-parallel over the N point dimension across 8 cores; each
core reduces its 524288-point shard to per-partition partials [128, 2*NT]
(per-chunk sum(mask*q^2) and count columns); the host sums 8*128*NT partials,
takes sqrt and divides.

The host packs each core's shard chunk-major and component-separated
([u0|v1|w2|u1|v0|u2|w0|w1|v2|sd] per chunk, partition-major inside each
block).  That makes every chunk ONE contiguous 2-4MB DMA (~97% of the
358GB/s per-core HBM roofline) and every SBUF read contiguous (no stride-3
penalty, wide fused ops).  Per chunk (F points/partition):
  VectorE: 3 f32 shear adds, mask via tensor_scalar(is_lt) with fused
           row-sum accum (= count, free), cross products factored as
           p1*(p2+p3) + p2*p3 on pre-scaled bf16 copies (2x mode), a
           3-level wide bf16 fold of the 8 weighted terms, q*m
  ScalarE: pre-scaled copies p12|p3 (alpha-factorization of the cross
           weights, a1==a2 so u0|v1 share one wide copy), weighted squares
           as wide activation(Square, scale) ops, final Square(q*m) with
           accum_out -> per-partition sum(mask*q^2)
  chunks tapered [512,1024,1024,1024,512] so the first compute starts early
  and the final serial chain after the last DMA is short.
Measured ~78.7us/core on TRN2 vs ~56us pure-DMA roofline (fixed NEFF
preamble + drain/barrier tail account for most of the difference).
"""

import numpy as np

N = 4_194_304
NCORES = 8
N_LOCAL = N // NCORES  # 524288
P = 128
J = N_LOCAL // P  # 4096 points per partition (partition-major layout)
# chunk widths; tapered head (compute starts sooner) and tail (short final
# serial chain)
CHUNKS = [512, 1024, 1024, 1024, 512]
NT = len(CHUNKS)
assert sum(CHUNKS) == J

THRESH = 1e-8


def _weights():
    vp, Ep = 0.4, 0.21
    Ci = np.zeros((6, 6), dtype=np.float64)
    Ci[0, 0] = 1 / Ep;  Ci[0, 1] = -vp / Ep; Ci[0, 2] = -vp / Ep
    Ci[1, 0] = -vp / Ep; Ci[1, 1] = 1 / Ep;  Ci[1, 2] = -vp / Ep
    Ci[2, 0] = -vp;      Ci[2, 1] = -vp;     Ci[2, 2] = 1 / Ep
    Ci[3, 3] = 2 * (1 + vp) / Ep
    Ci[4, 4] = Ci[3, 3]
    Ci[5, 5] = Ci[3, 3]
    # match reference: inverse computed in f64, cast to f32
    C = np.linalg.inv(Ci).astype(np.float32).astype(np.float64)
    Cs = 0.5 * (C + C.T)
    A = Cs[:3, :3]
    d = 0.25 * Cs[3, 3]
    return dict(
        w11=A[0, 0], w22=A[1, 1], w33=A[2, 2],
        w12=2 * A[0, 1], w13=2 * A[0, 2], w23=2 * A[1, 2],
        d=d,
    )


_NC = None


def _build_nc():
    import concourse.bacc as bacc
    import concourse.mybir as mybir
    import concourse.tile as tile

    W = _weights()
    rd = float(np.sqrt(W["d"]))
    # factor cross weights: w12 = a1*a2, w13 = a1*a3, w23 = a2*a3 so the
    # cross products use pre-scaled bf16 copies p_i = a_i*s_i (all-bf16 ->
    # DVE 2x mode); a1 == a2 and w11 == w22 for these constants, so u0|v1
    # share one wide scaled copy and one wide square.
    a1s = float(np.sqrt(W["w12"] * W["w13"] / W["w23"]))
    a2s = float(W["w12"] / a1s)
    a3s = float(W["w13"] / a1s)
    assert abs(a1s - a2s) < 1e-12 and abs(W["w11"] - W["w22"]) < 1e-12
    rz12f = float(np.sqrt(W["w11"]) / a1s)  # z12 = Sq(p12 * rz12f)
    rz3f = float(np.sqrt(W["w33"]) / a3s)   # z3  = Sq(p3 * rz3f)

    f32 = mybir.dt.float32
    bf16 = mybir.dt.bfloat16
    Sq = mybir.ActivationFunctionType.Square
    ALU = mybir.AluOpType

    nc = bacc.Bacc()
    # host packs each core's shard chunk-major: for each chunk t, partition p:
    # [u (3F interleaved) | v (3F) | w (3F) | sd (F)] -> one contiguous DMA
    # per chunk (4MB-class, ~97% DMA efficiency)
    packed = nc.dram_tensor("packed", [P, 10 * J], f32, kind="ExternalInput")
    out = nc.dram_tensor("out", [P, 2 * NT], f32, kind="ExternalOutput")

    with tile.TileContext(nc) as tc:
        with (
            tc.tile_pool(name="io", bufs=2) as io,
            tc.tile_pool(name="mid", bufs=3) as mid,
            tc.tile_pool(name="stats", bufs=1) as stats_pool,
        ):
            stats = stats_pool.tile([P, 2 * NT], f32)

            c0 = 0
            for t, F in enumerate(CHUNKS):
                buf = io.tile([P, 10 * F], f32, tag="buf")
                nc.sync.dma_start(out=buf[:], in_=packed[:, c0:c0 + 10 * F])
                c0 += 10 * F

                # host-packed chunk layout (all contiguous [P, F] blocks):
                # [u0 v1 w2 | u1 v0 u2 w0 w1 v2 | sd]
                u0v1 = buf[:, 0 * F:2 * F]
                w2 = buf[:, 2 * F:3 * F]
                u1, v0 = buf[:, 3 * F:4 * F], buf[:, 4 * F:5 * F]
                u2, w0 = buf[:, 5 * F:6 * F], buf[:, 6 * F:7 * F]
                w1, v2 = buf[:, 7 * F:8 * F], buf[:, 8 * F:9 * F]
                sd = buf[:, 9 * F:10 * F]

                # shear strain components into one [P,3F] tile
                # (f32 contiguous in, bf16 out)
                s456 = mid.tile([P, 3 * F], bf16, tag="s456")
                nc.vector.tensor_add(s456[:, 0:F], u1, v0)
                nc.vector.tensor_add(s456[:, F:2 * F], u2, w0)
                nc.vector.tensor_add(s456[:, 2 * F:3 * F], w1, v2)

                # pre-scaled bf16 copies on ScalarE (alpha1 == alpha2, so
                # u0 and v1 share one 2F-wide copy)
                p12 = mid.tile([P, 2 * F], bf16, tag="p12")
                p3 = mid.tile([P, F], bf16, tag="p3")
                nc.scalar.mul(p12, u0v1, a1s)
                nc.scalar.mul(p3, w2, a3s)

                # mask (f32 single-src 2x); fused row-sum accum = count
                m = mid.tile([P, F], bf16, tag="m")
                nc.vector.tensor_scalar(
                    out=m, in0=sd, scalar1=THRESH, scalar2=None, op0=ALU.is_lt,
                    op1=ALU.add, accum_out=stats[:, NT + t:NT + t + 1])

                # term tiles: X = [z4 z5 z6 | z3], Y1 = [z1 z2], Y2 = [ca cb]
                X = mid.tile([P, 4 * F], bf16, tag="X")
                Y1 = mid.tile([P, 2 * F], bf16, tag="Y1")
                Y2 = mid.tile([P, 2 * F], bf16, tag="Y2")

                # weighted squares on ScalarE (wide ops; shared scales)
                nc.scalar.activation(X[:, 0:3 * F], s456, Sq, scale=rd)
                nc.scalar.activation(X[:, 3 * F:4 * F], p3, Sq, scale=rz3f)
                nc.scalar.activation(Y1, p12, Sq, scale=rz12f)

                # cross products, factored: p1p2 + p1p3 + p2p3 =
                # p1*(p2+p3) + p2*p3  (all bf16, DVE 2x)
                tp = mid.tile([P, F], bf16, tag="tp")
                nc.vector.tensor_add(tp, p12[:, F:2 * F], p3)
                nc.vector.tensor_mul(Y2[:, 0:F], p12[:, 0:F], tp)
                nc.vector.tensor_mul(Y2[:, F:2 * F], p12[:, F:2 * F], p3)

                # combine 8 terms with a 3-level wide fold (work 7F, 4 ops)
                nc.vector.tensor_add(Y1, Y1, Y2)                    # 2F
                nc.vector.tensor_add(X[:, 0:2 * F], X[:, 0:2 * F],
                                     X[:, 2 * F:4 * F])             # 2F
                nc.vector.tensor_add(Y1, Y1, X[:, 0:2 * F])         # 2F
                q = p3  # reuse consumed tile for q
                nc.vector.tensor_add(q, Y1[:, 0:F], Y1[:, F:2 * F])  # F

                # qm = q * mask (bf16 2x), then ssq via fused square+row-sum
                nc.vector.tensor_mul(m, q, m)
                junk1 = mid.tile([P, F], bf16, tag="junk1")
                nc.scalar.activation(
                    junk1, m, Sq, accum_out=stats[:, t:t + 1])

            nc.sync.dma_start(out=out[:, :], in_=stats[:])

    nc.compile()
    return nc


def _get_nc():
    global _NC
    if _NC is None:
        _NC = _build_nc()
    return _NC


def _run(in_maps, trace=False, **kwargs):
    from concourse.bass_utils import run_bass_kernel_spmd

    nc = _get_nc()
    return run_bass_kernel_spmd(
        nc, in_maps, core_ids=list(range(NCORES)), trace=trace, **kwargs)


def _make_in_maps(grad_u, grad_v, grad_w, gt_sdf):
    grad_u = np.asarray(grad_u, dtype=np.float32)
    grad_v = np.asarray(grad_v, dtype=np.float32)
    grad_w = np.asarray(grad_w, dtype=np.float32)
    gt_sdf = np.asarray(gt_sdf, dtype=np.float32)
    in_maps = []
    for c in range(NCORES):
        sl = slice(c * N_LOCAL, (c + 1) * N_LOCAL)
        gu = grad_u[sl].reshape(P, J, 3)
        gv = grad_v[sl].reshape(P, J, 3)
        gw = grad_w[sl].reshape(P, J, 3)
        sd = gt_sdf[sl].reshape(P, J)
        parts = []
        off = 0
        for F in CHUNKS:
            s = slice(off, off + F)
            parts += [gu[:, s, 0], gv[:, s, 1], gw[:, s, 2],
                      gu[:, s, 1], gv[:, s, 0],
                      gu[:, s, 2], gw[:, s, 0],
                      gw[:, s, 1], gv[:, s, 2],
                      sd[:, s]]
            off += F
        packed = np.ascontiguousarray(np.concatenate(parts, axis=1))
        in_maps.append({"packed": packed})
    return in_maps


def _finalize(results):
    ssq = 0.0
    cnt = 0.0
    for res in results:
        st = np.asarray(res["out"], dtype=np.float64)
        ssq += st[:, :NT].sum()
        cnt += st[:, NT:].sum()
    Wv = np.sqrt(ssq)
    return np.float32(Wv / cnt)


def kernel(grad_u, grad_v, grad_w, gt_sdf):
    in_maps = _make_in_maps(grad_u, grad_v, grad_w, gt_sdf)
    res = _run(in_maps, trace=False)
    return _finalize(res.results)



# revision 9
# speedup vs baseline: 1.0978x; 1.0978x over previous
"""Trainium2 Bass kernel for nn_BiomechanicsLoss (masked quadratic-form loss).

Math (per point): et = [u0, v1, w2, .5(u1+v0), .5(u2+w0), .5(w1+v2)],
q = et^T C et with C = inv(compliance) cast to f32.  Loss =
sqrt(sum_masked(q^2)) / count_masked, mask = gt_sdf < 1e-8.

q = et^T sym(C) et and sym(C) is block-diagonal: a 3x3 normal block A over
s1,s2,s3 = u0,v1,w2 plus a scalar d = 0.25*Cs[3,3] times (s4^2+s5^2+s6^2)
with s4 = u1+v0, s5 = u2+w0, s6 = w1+v2.  LDL^T of A gives a pure
sum-of-squares form:
  q = l1*(s1 + a12 s2 + a13 s3)^2 + l2*(s2 + a23 s3)^2 + l3*s3^2
      + d*(s4^2 + s5^2 + s6^2)
so every weighted square runs on ScalarE as one activation(Square, scale)
with the sqrt of the weight folded into the free scale.

Sharding: pure data-parallel over N across 8 cores; each core reduces its
524288-point shard to per-partition partials [128, 2*NT] (per-chunk
sum(mask*q^2) and count columns); the host sums partials, sqrts, divides.

The host packs each core's shard chunk-major, component-separated AND
converted to bf16 ([u1 u2 | v0 w0 | s1 s2 s3 | w1 v2 | sd] per chunk,
partition-major inside each block).  bf16 halves HBM traffic vs f32
(10.5MB/core, ~30us at the ~340GB/s per-core DMA rate) and puts every
elementwise op in the DVE 2x/4x fast modes.  Per chunk the work is spread
so each engine stays under the DMA shadow:
  DVE:    s45 = u12+vw (one wide add), mask via tensor_scalar(is_lt) with
          fused row-sum accum (= count), t1/t2/t3 scaled copies
          (tensor_scalar 4x), y1 assembly, 3 fold adds
  Pool:   s6 = w1+v2, y2 = s2+t3, qm = q*m (plain tensor_tensor only --
          the Pool engine rejects TensorScalarPtr at compile)
  ScalarE: the 4 weighted Square activations (one wide over s456) and the
          final Square(q*m) with accum_out -> per-partition sum(mask*q^2)
"""

import numpy as np

N = 4_194_304
NCORES = 8
N_LOCAL = N // NCORES  # 524288
P = 128
J = N_LOCAL // P  # 4096 points per partition (partition-major layout)
# chunk widths: large middle for low per-op overhead, small tail so the
# serial chain after the last DMA is short
CHUNKS = [1536, 2048, 512]
NT = len(CHUNKS)
assert sum(CHUNKS) == J

THRESH = 1e-8


def _ldl():
    vp, Ep = 0.4, 0.21
    Ci = np.zeros((6, 6), dtype=np.float64)
    Ci[0, 0] = 1 / Ep;  Ci[0, 1] = -vp / Ep; Ci[0, 2] = -vp / Ep
    Ci[1, 0] = -vp / Ep; Ci[1, 1] = 1 / Ep;  Ci[1, 2] = -vp / Ep
    Ci[2, 0] = -vp;      Ci[2, 1] = -vp;     Ci[2, 2] = 1 / Ep
    Ci[3, 3] = 2 * (1 + vp) / Ep
    Ci[4, 4] = Ci[3, 3]
    Ci[5, 5] = Ci[3, 3]
    # match reference: inverse computed in f64, cast to f32
    C = np.linalg.inv(Ci).astype(np.float32).astype(np.float64)
    Cs = 0.5 * (C + C.T)
    A = Cs[:3, :3]
    d = 0.25 * Cs[3, 3]
    l1 = A[0, 0]
    a12 = A[0, 1] / l1
    a13 = A[0, 2] / l1
    A2 = A[1:, 1:] - np.outer(A[0, 1:], A[0, 1:]) / l1
    l2 = A2[0, 0]
    a23 = A2[0, 1] / l2
    l3 = A2[1, 1] - A2[0, 1] ** 2 / l2
    return dict(
        a12=float(a12), a13=float(a13), a23=float(a23),
        rl1=float(np.sqrt(l1)), rl2=float(np.sqrt(l2)),
        rl3=float(np.sqrt(l3)), rd=float(np.sqrt(d)),
    )


_NC = None


def _build_nc():
    import concourse.bacc as bacc
    import concourse.mybir as mybir
    import concourse.tile as tile

    W = _ldl()

    f32 = mybir.dt.float32
    bf16 = mybir.dt.bfloat16
    Sq = mybir.ActivationFunctionType.Square
    ALU = mybir.AluOpType

    nc = bacc.Bacc()
    # host packs each core's shard chunk-major, bf16: for each chunk t:
    # [u1 u2 | v0 w0 | s1 s2 s3 | w1 v2 | sd] -> one contiguous DMA per chunk
    packed = nc.dram_tensor("packed", [P, 10 * J], bf16, kind="ExternalInput")
    out = nc.dram_tensor("out", [P, 2 * NT], f32, kind="ExternalOutput")

    with tile.TileContext(nc) as tc:
        with (
            tc.tile_pool(name="io", bufs=2) as io,
            tc.tile_pool(name="mid", bufs=2) as mid,
            tc.tile_pool(name="stats", bufs=1) as stats_pool,
        ):
            stats = stats_pool.tile([P, 2 * NT], f32)

            c0 = 0
            for t, F in enumerate(CHUNKS):
                buf = io.tile([P, 10 * F], bf16, tag="buf")
                nc.sync.dma_start(out=buf[:], in_=packed[:, c0:c0 + 10 * F])
                c0 += 10 * F

                u12 = buf[:, 0 * F:2 * F]
                vw = buf[:, 2 * F:4 * F]
                s1 = buf[:, 4 * F:5 * F]
                s2 = buf[:, 5 * F:6 * F]
                s3 = buf[:, 6 * F:7 * F]
                w1 = buf[:, 7 * F:8 * F]
                v2 = buf[:, 8 * F:9 * F]
                sd = buf[:, 9 * F:10 * F]

                # shear strains [s4 s5 s6] in one tile; s4/s5 as one wide
                # DVE add, s6 on Pool
                s456 = mid.tile([P, 3 * F], bf16, tag="s456")
                nc.vector.tensor_add(s456[:, 0:2 * F], u12, vw)
                nc.gpsimd.tensor_add(s456[:, 2 * F:3 * F], w1, v2)

                # mask on DVE (tensor_scalar 4x); fused row-sum accum = count
                m = mid.tile([P, F], bf16, tag="m")
                nc.vector.tensor_scalar(
                    out=m, in0=sd, scalar1=THRESH, scalar2=None, op0=ALU.is_lt,
                    op1=ALU.add, accum_out=stats[:, NT + t:NT + t + 1])

                # y1 = s1 + a12*s2 + a13*s3; y2 = s2 + a23*s3
                # scaled copies on DVE (tensor_scalar 4x), y2 add on Pool;
                # t1 becomes uu in place, y1 lands in t2's slot (dead by then)
                t1 = mid.tile([P, F], bf16, tag="t1")
                t2 = mid.tile([P, F], bf16, tag="t2")
                t3 = mid.tile([P, F], bf16, tag="t3")
                nc.vector.tensor_scalar_mul(t1, s2, W["a12"])
                nc.vector.tensor_scalar_mul(t2, s3, W["a13"])
                nc.vector.tensor_scalar_mul(t3, s3, W["a23"])
                y2 = mid.tile([P, F], bf16, tag="y2")
                nc.gpsimd.tensor_add(y2, s2, t3)
                nc.vector.tensor_add(t1, t1, t2)  # uu
                y1 = t2
                nc.vector.tensor_add(y1, s1, t1)

                # weighted squares on ScalarE: X = [z4 z5 z6 | zy1 zy2 zy3]
                X = mid.tile([P, 6 * F], bf16, tag="X")
                nc.scalar.activation(X[:, 0:3 * F], s456, Sq, scale=W["rd"])
                nc.scalar.activation(X[:, 3 * F:4 * F], y1, Sq, scale=W["rl1"])
                nc.scalar.activation(X[:, 4 * F:5 * F], y2, Sq, scale=W["rl2"])
                nc.scalar.activation(X[:, 5 * F:6 * F], s3, Sq, scale=W["rl3"])

                # fold 6 terms -> q (3 wide DVE adds)
                nc.vector.tensor_add(X[:, 0:3 * F], X[:, 0:3 * F],
                                     X[:, 3 * F:6 * F])
                q = mid.tile([P, F], bf16, tag="q")
                nc.vector.tensor_add(q, X[:, 0:F], X[:, F:2 * F])
                nc.vector.tensor_add(q, q, X[:, 2 * F:3 * F])

                # qm = q*mask on Pool (into t3's dead slot), then ssq via
                # fused square+row-sum on ScalarE (output into t1's dead slot)
                qm = t3
                nc.gpsimd.tensor_mul(qm, q, m)
                nc.scalar.activation(
                    t1, qm, Sq, accum_out=stats[:, t:t + 1])

            nc.sync.dma_start(out=out[:, :], in_=stats[:])

    nc.compile()
    return nc


def _get_nc():
    global _NC
    if _NC is None:
        _NC = _build_nc()
    return _NC


def _run(in_maps, trace=False, **kwargs):
    from concourse.bass_utils import run_bass_kernel_spmd

    nc = _get_nc()
    return run_bass_kernel_spmd(
        nc, in_maps, core_ids=list(range(NCORES)), trace=trace, **kwargs)


def _make_in_maps(grad_u, grad_v, grad_w, gt_sdf):
    import ml_dtypes

    bf = ml_dtypes.bfloat16
    grad_u = np.asarray(grad_u, dtype=np.float32).astype(bf)
    grad_v = np.asarray(grad_v, dtype=np.float32).astype(bf)
    grad_w = np.asarray(grad_w, dtype=np.float32).astype(bf)
    gt_sdf = np.asarray(gt_sdf, dtype=np.float32).astype(bf)
    in_maps = []
    for c in range(NCORES):
        sl = slice(c * N_LOCAL, (c + 1) * N_LOCAL)
        gu = grad_u[sl].reshape(P, J, 3)
        gv = grad_v[sl].reshape(P, J, 3)
        gw = grad_w[sl].reshape(P, J, 3)
        sd = gt_sdf[sl].reshape(P, J)
        parts = []
        off = 0
        for F in CHUNKS:
            s = slice(off, off + F)
            parts += [gu[:, s, 1], gu[:, s, 2],     # u1 u2
                      gv[:, s, 0], gw[:, s, 0],     # v0 w0
                      gu[:, s, 0], gv[:, s, 1], gw[:, s, 2],  # s1 s2 s3
                      gw[:, s, 1], gv[:, s, 2],     # w1 v2
                      sd[:, s]]
            off += F
        packed = np.ascontiguousarray(np.concatenate(parts, axis=1))
        in_maps.append({"packed": packed})
    return in_maps


def _finalize(results):
    ssq = 0.0
    cnt = 0.0
    for res in results:
        st = np.asarray(res["out"], dtype=np.float64)
        ssq += st[:, :NT].sum()
        cnt += st[:, NT:].sum()
    Wv = np.sqrt(ssq)
    return np.float32(Wv / cnt)


def kernel(grad_u, grad_v, grad_w, gt_sdf):
    in_maps = _make_in_maps(grad_u, grad_v, grad_w, gt_sdf)
    res = _run(in_maps, trace=False)
    return _finalize(res.results)


# revision 10
# speedup vs baseline: 1.1740x; 1.0694x over previous
"""Trainium2 Bass kernel for nn_BiomechanicsLoss (masked quadratic-form loss).

Math (per point): et = [u0, v1, w2, .5(u1+v0), .5(u2+w0), .5(w1+v2)],
q = et^T C et with C = inv(compliance) cast to f32.  Loss =
sqrt(sum_masked(q^2)) / count_masked, mask = gt_sdf < 1e-8.

q = et^T sym(C) et; sym(C) is block-diagonal: 3x3 normal block A over
s1,s2,s3 = u0,v1,w2 plus d = 0.25*Cs[3,3] times (s4^2+s5^2+s6^2) with
s4 = u1+v0, s5 = u2+w0, s6 = w1+v2.  The constants satisfy w11 == w22 and
w13 == w23, so the polarization identity gives a cheap sum-of-squares:
  crosses = P'^2 - (p1'^2+p2'^2+p3'^2),  P' = p1'+p2'+p3', p_i' = a_i/sqrt2*s_i
  q = P'^2 + b12*(s1^2+s2^2) + b3*s3^2 + d*(s4^2+s5^2+s6^2)
with a1 = a2 = sqrt(w12), a3 = w13/a1, b12 = w11 - w12/2, b3 = w33 - a3^2/2
(all positive).  Every weighted square runs on ScalarE as one
activation(Square, scale); a1 == a2 makes p12' one wide tensor_scalar and
b12's square one wide activation.

Sharding: pure data-parallel over N across 8 cores; each core reduces its
524288-point shard to per-partition partials [128, 2*NT]; host sums, sqrts,
divides.

The host packs each core's shard chunk-major, component-separated, bf16:
[u1 u2 w1 | v0 w0 v2 | s1 s2 | s3 | sd] per chunk.  bf16 halves HBM traffic
(10.5MB/core, ~27us DMA) and enables DVE 2x/4x modes.  The adjacent u/v
blocks make ALL THREE shear adds one wide DVE tensor_tensor.  Emission is
software-pipelined (stage A of chunk t+1 before stage B of chunk t) so no
engine queue stalls on cross-engine dependencies:
  stage A: DVE mask/p12'/P'/s456, Pool zP = P'^2, ScalarE 3 wide squares
  stage B: DVE folds + q*m (Pool pre-folds the zb3+z6 pair), ScalarE
           Square(q*m) with accum_out -> per-partition sum(mask*q^2)
"""

import numpy as np

N = 4_194_304
NCORES = 8
N_LOCAL = N // NCORES  # 524288
P = 128
J = N_LOCAL // P  # 4096 points per partition (partition-major layout)
CHUNKS = [768, 768, 768, 768, 512, 512]
NT = len(CHUNKS)
assert sum(CHUNKS) == J

THRESH = 1e-8


def _weights():
    vp, Ep = 0.4, 0.21
    Ci = np.zeros((6, 6), dtype=np.float64)
    Ci[0, 0] = 1 / Ep;  Ci[0, 1] = -vp / Ep; Ci[0, 2] = -vp / Ep
    Ci[1, 0] = -vp / Ep; Ci[1, 1] = 1 / Ep;  Ci[1, 2] = -vp / Ep
    Ci[2, 0] = -vp;      Ci[2, 1] = -vp;     Ci[2, 2] = 1 / Ep
    Ci[3, 3] = 2 * (1 + vp) / Ep
    Ci[4, 4] = Ci[3, 3]
    Ci[5, 5] = Ci[3, 3]
    # match reference: inverse computed in f64, cast to f32
    C = np.linalg.inv(Ci).astype(np.float32).astype(np.float64)
    Cs = 0.5 * (C + C.T)
    A = Cs[:3, :3]
    d = 0.25 * Cs[3, 3]
    w11, w33 = A[0, 0], A[2, 2]
    w12, w13, w23 = 2 * A[0, 1], 2 * A[0, 2], 2 * A[1, 2]
    assert abs(A[0, 0] - A[1, 1]) < 1e-12 and abs(w13 - w23) < 1e-12
    a1 = np.sqrt(w12)
    a3 = w13 / a1
    b12 = w11 - w12 / 2
    b3 = w33 - a3 * a3 / 2
    assert b12 > 0 and b3 > 0
    return dict(
        a12s=float(a1 / np.sqrt(2)), a3s=float(a3 / np.sqrt(2)),
        rb12=float(np.sqrt(b12)), rb3=float(np.sqrt(b3)),
        rd=float(np.sqrt(d)),
    )


_NC = None


def _build_nc():
    import concourse.bacc as bacc
    import concourse.mybir as mybir
    import concourse.tile as tile

    W = _weights()

    f32 = mybir.dt.float32
    bf16 = mybir.dt.bfloat16
    Sq = mybir.ActivationFunctionType.Square
    ALU = mybir.AluOpType

    nc = bacc.Bacc()
    packed = nc.dram_tensor("packed", [P, 10 * J], bf16, kind="ExternalInput")
    out = nc.dram_tensor("out", [P, 2 * NT], f32, kind="ExternalOutput")

    with tile.TileContext(nc) as tc:
        with (
            tc.tile_pool(name="io", bufs=3) as io,
            tc.tile_pool(name="mid", bufs=2) as mid,
            tc.tile_pool(name="stats", bufs=1) as stats_pool,
        ):
            stats = stats_pool.tile([P, 2 * NT], f32)

            offs = np.cumsum([0] + [10 * F for F in CHUNKS]).tolist()
            ctx = {}

            def stage_a(t):
                F = CHUNKS[t]
                buf = io.tile([P, 10 * F], bf16, tag="buf")
                nc.sync.dma_start(
                    out=buf[:], in_=packed[:, offs[t]:offs[t + 1]])
                s12 = buf[:, 6 * F:8 * F]
                s3 = buf[:, 8 * F:9 * F]
                sd = buf[:, 9 * F:10 * F]

                # mask (fused count accum) and P' = p1'+p2'+p3' on DVE
                m = mid.tile([P, F], bf16, tag="m")
                nc.vector.tensor_scalar(
                    out=m, in0=sd, scalar1=THRESH, scalar2=None, op0=ALU.is_lt,
                    op1=ALU.add, accum_out=stats[:, NT + t:NT + t + 1])
                p12 = mid.tile([P, 2 * F], bf16, tag="p12")
                nc.vector.tensor_scalar_mul(p12, s12, W["a12s"])
                tP = mid.tile([P, F], bf16, tag="tP")
                nc.vector.tensor_add(tP, p12[:, 0:F], p12[:, F:2 * F])
                Pv = p12[:, 0:F]  # dead slot reuse
                nc.vector.scalar_tensor_tensor(
                    Pv, s3, W["a3s"], tP, ALU.mult, ALU.add)
                # all three shear adds in ONE wide op
                s456 = mid.tile([P, 3 * F], bf16, tag="s456")
                nc.vector.tensor_add(s456, buf[:, 0:3 * F], buf[:, 3 * F:6 * F])

                # zP = P'^2 on Pool (off the fold critical path)
                zP = mid.tile([P, F], bf16, tag="zP")
                nc.gpsimd.tensor_mul(zP, Pv, Pv)

                # squares on ScalarE: X = [zb12 zb3 | z456]; the two
                # buf-dependent ones first so ScalarE never waits on DVE
                X = mid.tile([P, 6 * F], bf16, tag="X")
                nc.scalar.activation(X[:, 0:2 * F], s12, Sq, scale=W["rb12"])
                nc.scalar.activation(X[:, 2 * F:3 * F], s3, Sq, scale=W["rb3"])
                nc.scalar.activation(X[:, 3 * F:6 * F], s456, Sq, scale=W["rd"])
                ctx[t] = (F, m, tP, X, zP, p12)

            def stage_b(t):
                F, m, tP, X, zP, p12 = ctx.pop(t)
                # fold 7 terms: [zb1+z4, zb2+z5] wide on DVE, zb3+z6 on Pool
                nc.vector.tensor_add(X[:, 0:2 * F], X[:, 0:2 * F],
                                     X[:, 3 * F:5 * F])
                a2 = mid.tile([P, F], bf16, tag="a2")
                nc.gpsimd.tensor_add(a2, X[:, 2 * F:3 * F], X[:, 5 * F:6 * F])
                q = mid.tile([P, F], bf16, tag="q")
                nc.vector.tensor_add(q, X[:, 0:F], X[:, F:2 * F])
                nc.vector.tensor_add(q, q, zP)
                nc.vector.tensor_add(q, q, a2)
                qm = tP  # dead slot reuse
                nc.vector.tensor_mul(qm, q, m)
                nc.scalar.activation(
                    p12[:, F:2 * F], qm, Sq, accum_out=stats[:, t:t + 1])

            # software pipeline: A(0) A(1) B(0) A(2) B(1) ... B(last)
            stage_a(0)
            for t in range(1, NT):
                stage_a(t)
                stage_b(t - 1)
            stage_b(NT - 1)

            nc.sync.dma_start(out=out[:, :], in_=stats[:])

    nc.compile()
    return nc


def _get_nc():
    global _NC
    if _NC is None:
        _NC = _build_nc()
    return _NC


def _run(in_maps, trace=False, **kwargs):
    from concourse.bass_utils import run_bass_kernel_spmd

    nc = _get_nc()
    return run_bass_kernel_spmd(
        nc, in_maps, core_ids=list(range(NCORES)), trace=trace, **kwargs)


def _make_in_maps(grad_u, grad_v, grad_w, gt_sdf):
    import ml_dtypes

    bf = ml_dtypes.bfloat16
    grad_u = np.asarray(grad_u, dtype=np.float32).astype(bf)
    grad_v = np.asarray(grad_v, dtype=np.float32).astype(bf)
    grad_w = np.asarray(grad_w, dtype=np.float32).astype(bf)
    gt_sdf = np.asarray(gt_sdf, dtype=np.float32).astype(bf)
    in_maps = []
    for c in range(NCORES):
        sl = slice(c * N_LOCAL, (c + 1) * N_LOCAL)
        gu = grad_u[sl].reshape(P, J, 3)
        gv = grad_v[sl].reshape(P, J, 3)
        gw = grad_w[sl].reshape(P, J, 3)
        sd = gt_sdf[sl].reshape(P, J)
        parts = []
        off = 0
        for F in CHUNKS:
            s = slice(off, off + F)
            parts += [gu[:, s, 1], gu[:, s, 2], gw[:, s, 1],   # u1 u2 w1
                      gv[:, s, 0], gw[:, s, 0], gv[:, s, 2],   # v0 w0 v2
                      gu[:, s, 0], gv[:, s, 1],                # s1 s2
                      gw[:, s, 2],                             # s3
                      sd[:, s]]
            off += F
        packed = np.ascontiguousarray(np.concatenate(parts, axis=1))
        in_maps.append({"packed": packed})
    return in_maps


def _finalize(results):
    ssq = 0.0
    cnt = 0.0
    for res in results:
        st = np.asarray(res["out"], dtype=np.float64)
        ssq += st[:, :NT].sum()
        cnt += st[:, NT:].sum()
    Wv = np.sqrt(ssq)
    return np.float32(Wv / cnt)


def kernel(grad_u, grad_v, grad_w, gt_sdf):
    in_maps = _make_in_maps(grad_u, grad_v, grad_w, gt_sdf)
    res = _run(in_maps, trace=False)
    return _finalize(res.results)


# revision 12
# speedup vs baseline: 1.5943x; 1.3580x over previous
"""Trainium2 Bass kernel for nn_BiomechanicsLoss (masked quadratic-form loss).

Math (per point): et = [u0, v1, w2, .5(u1+v0), .5(u2+w0), .5(w1+v2)],
q = et^T C et with C = inv(compliance) cast to f32.  Loss =
sqrt(sum_masked(q^2)) / count_masked, mask = gt_sdf < 1e-8.

q = et^T sym(C) et; sym(C) is block-diagonal: 3x3 normal block A over
s1,s2,s3 = u0,v1,w2 plus d = 0.25*Cs[3,3] times (s4^2+s5^2+s6^2) with
s4 = u1+v0, s5 = u2+w0, s6 = w1+v2.  The constants satisfy w11 == w22 and
w13 == w23, so polarization gives a pure sum of 7 squares:
  q = P'^2 + b12*(s1^2+s2^2) + b3*s3^2 + d*(s4^2+s5^2+s6^2)
  P' = (a1/sqrt2)(s1+s2) + (a3/sqrt2) s3,  a1 = sqrt(w12), a3 = w13/a1,
  b12 = w11 - w12/2, b3 = w33 - a3^2/2   (all positive).

Engine split (per 512-point-per-partition chunk, software-pipelined):
  DVE:   mask via tensor_scalar(is_lt) with fused count accum, p12' wide
         scaled copy (tensor_scalar 4x; a1 == a2 so one op covers s1,s2),
         s456 = ONE wide add (host packs [u1 u2 w1 | v0 w0 v2]),
         P' via scalar_tensor_tensor, final masked reduce via native
         tensor_tensor_reduce(zq*m -> accum)
  Pool:  tP = p1'+p2', zP = P'^2 (plain tensor_tensor only)
  ScalarE: the 3 weighted Square activations (wide: zb12 2F, zb3 F,
         z456 3F) and zq = Square(q) read straight from PSUM
  PE:    THE FOLD -- 7 identity-weight matmuls accumulate the 7 square
         terms into one PSUM bank (q per point), freeing DVE of the
         fold-adds entirely.
Host packs bf16 (halves HBM traffic; ~27us DMA at ~340GB/s/core) and the
host sums the [128, 2*NT] per-core partials, sqrts, divides.
"""

import numpy as np

N = 4_194_304
NCORES = 8
N_LOCAL = N // NCORES  # 524288
P = 128
J = N_LOCAL // P  # 4096 points per partition (partition-major layout)
F = 512  # chunk width == one PSUM bank of f32
CHUNKS = [F] * (J // F)  # 8 chunks
NT = len(CHUNKS)

THRESH = 1e-8


def _weights():
    vp, Ep = 0.4, 0.21
    Ci = np.zeros((6, 6), dtype=np.float64)
    Ci[0, 0] = 1 / Ep;  Ci[0, 1] = -vp / Ep; Ci[0, 2] = -vp / Ep
    Ci[1, 0] = -vp / Ep; Ci[1, 1] = 1 / Ep;  Ci[1, 2] = -vp / Ep
    Ci[2, 0] = -vp;      Ci[2, 1] = -vp;     Ci[2, 2] = 1 / Ep
    Ci[3, 3] = 2 * (1 + vp) / Ep
    Ci[4, 4] = Ci[3, 3]
    Ci[5, 5] = Ci[3, 3]
    # match reference: inverse computed in f64, cast to f32
    C = np.linalg.inv(Ci).astype(np.float32).astype(np.float64)
    Cs = 0.5 * (C + C.T)
    A = Cs[:3, :3]
    d = 0.25 * Cs[3, 3]
    w11, w33 = A[0, 0], A[2, 2]
    w12, w13, w23 = 2 * A[0, 1], 2 * A[0, 2], 2 * A[1, 2]
    assert abs(A[0, 0] - A[1, 1]) < 1e-12 and abs(w13 - w23) < 1e-12
    a1 = np.sqrt(w12)
    a3 = w13 / a1
    b12 = w11 - w12 / 2
    b3 = w33 - a3 * a3 / 2
    assert b12 > 0 and b3 > 0
    return dict(
        a12s=float(a1 / np.sqrt(2)), a3s=float(a3 / np.sqrt(2)),
        rb12=float(np.sqrt(b12)), rb3=float(np.sqrt(b3)),
        rd=float(np.sqrt(d)),
    )


_NC = None


def _build_nc():
    import concourse.bacc as bacc
    import concourse.mybir as mybir
    import concourse.tile as tile
    from concourse import masks

    W = _weights()

    f32 = mybir.dt.float32
    bf16 = mybir.dt.bfloat16
    Sq = mybir.ActivationFunctionType.Square
    ALU = mybir.AluOpType

    nc = bacc.Bacc()
    packed = nc.dram_tensor("packed", [P, 10 * J], bf16, kind="ExternalInput")
    out = nc.dram_tensor("out", [P, 2 * NT], f32, kind="ExternalOutput")

    with tile.TileContext(nc) as tc:
        with (
            tc.tile_pool(name="singles", bufs=1) as singles,
            tc.tile_pool(name="io", bufs=4) as io,
            tc.tile_pool(name="mid", bufs=3) as mid,
            tc.tile_pool(name="ps", bufs=2, space="PSUM") as ps,
            tc.tile_pool(name="stats", bufs=1) as stats_pool,
        ):
            ident = singles.tile([P, P], bf16)
            masks.make_identity(nc, ident[:])
            stats = stats_pool.tile([P, 2 * NT], f32)

            actx = {}
            pctx = {}

            def stage_a(t):
                buf = io.tile([P, 10 * F], bf16, tag="buf")
                nc.sync.dma_start(
                    out=buf[:], in_=packed[:, t * 10 * F:(t + 1) * 10 * F])
                s12 = buf[:, 6 * F:8 * F]
                s3 = buf[:, 8 * F:9 * F]
                sd = buf[:, 9 * F:10 * F]

                m = mid.tile([P, F], bf16, tag="m")
                nc.vector.tensor_scalar(
                    out=m, in0=sd, scalar1=THRESH, scalar2=None, op0=ALU.is_lt,
                    op1=ALU.add, accum_out=stats[:, NT + t:NT + t + 1])
                p12 = mid.tile([P, 2 * F], bf16, tag="p12")
                nc.vector.tensor_scalar_mul(p12, s12, W["a12s"])
                s456 = mid.tile([P, 3 * F], bf16, tag="s456")
                nc.vector.tensor_add(s456, buf[:, 0:3 * F], buf[:, 3 * F:6 * F])

                tP = mid.tile([P, F], bf16, tag="tP")
                nc.gpsimd.tensor_add(tP, p12[:, 0:F], p12[:, F:2 * F])
                Pv = mid.tile([P, F], bf16, tag="Pv")
                nc.vector.scalar_tensor_tensor(
                    Pv, s3, W["a3s"], tP, ALU.mult, ALU.add)
                zP = mid.tile([P, F], bf16, tag="zP")
                nc.gpsimd.tensor_mul(zP, Pv, Pv)

                # weighted squares on ScalarE: X = [zb12 zb3 | z456]
                X = mid.tile([P, 6 * F], bf16, tag="X")
                nc.scalar.activation(X[:, 0:2 * F], s12, Sq, scale=W["rb12"])
                nc.scalar.activation(X[:, 2 * F:3 * F], s3, Sq, scale=W["rb3"])
                nc.scalar.activation(X[:, 3 * F:6 * F], s456, Sq, scale=W["rd"])
                actx[t] = (m, X, zP)

            def stage_pe(t):
                m, X, zP = actx.pop(t)
                # fold the 7 square terms into one PSUM bank via
                # identity-weight accumulating matmuls
                qp = ps.tile([P, F], f32)
                for k in range(6):
                    nc.tensor.matmul(qp[:], ident[:], X[:, k * F:(k + 1) * F],
                                     start=(k == 0), stop=False)
                nc.tensor.matmul(qp[:], ident[:], zP[:], start=False, stop=True)
                pctx[t] = (m, qp)

            def stage_b(t):
                m, qp = pctx.pop(t)
                qm = mid.tile([P, F], bf16, tag="qm")
                nc.vector.tensor_mul(qm, qp, m)
                junk = mid.tile([P, F], bf16, tag="junk")
                nc.scalar.activation(
                    junk, qm, Sq, accum_out=stats[:, t:t + 1])

            # software pipeline, 3 chunks in flight:
            # A(0) A(1) [A(2) PE(0) B(0)] [A(3) PE(1) B(1)] ...
            stage_a(0)
            stage_a(1)
            for t in range(NT):
                if t + 2 < NT:
                    stage_a(t + 2)
                stage_pe(t)
                stage_b(t)

            nc.sync.dma_start(out=out[:, :], in_=stats[:])

    nc.compile()
    return nc


def _get_nc():
    global _NC
    if _NC is None:
        _NC = _build_nc()
    return _NC


def _run(in_maps, trace=False, **kwargs):
    from concourse.bass_utils import run_bass_kernel_spmd

    nc = _get_nc()
    return run_bass_kernel_spmd(
        nc, in_maps, core_ids=list(range(NCORES)), trace=trace, **kwargs)


def _make_in_maps(grad_u, grad_v, grad_w, gt_sdf):
    import ml_dtypes

    bf = ml_dtypes.bfloat16
    grad_u = np.asarray(grad_u, dtype=np.float32).astype(bf)
    grad_v = np.asarray(grad_v, dtype=np.float32).astype(bf)
    grad_w = np.asarray(grad_w, dtype=np.float32).astype(bf)
    gt_sdf = np.asarray(gt_sdf, dtype=np.float32).astype(bf)
    in_maps = []
    for c in range(NCORES):
        sl = slice(c * N_LOCAL, (c + 1) * N_LOCAL)
        gu = grad_u[sl].reshape(P, J, 3)
        gv = grad_v[sl].reshape(P, J, 3)
        gw = grad_w[sl].reshape(P, J, 3)
        sd = gt_sdf[sl].reshape(P, J)
        parts = []
        off = 0
        for Fc in CHUNKS:
            s = slice(off, off + Fc)
            parts += [gu[:, s, 1], gu[:, s, 2], gw[:, s, 1],   # u1 u2 w1
                      gv[:, s, 0], gw[:, s, 0], gv[:, s, 2],   # v0 w0 v2
                      gu[:, s, 0], gv[:, s, 1],                # s1 s2
                      gw[:, s, 2],                             # s3
                      sd[:, s]]
            off += Fc
        packed = np.ascontiguousarray(np.concatenate(parts, axis=1))
        in_maps.append({"packed": packed})
    return in_maps


def _finalize(results):
    ssq = 0.0
    cnt = 0.0
    for res in results:
        st = np.asarray(res["out"], dtype=np.float64)
        ssq += st[:, :NT].sum()
        cnt += st[:, NT:].sum()
    Wv = np.sqrt(ssq)
    return np.float32(Wv / cnt)


def kernel(grad_u, grad_v, grad_w, gt_sdf):
    in_maps = _make_in_maps(grad_u, grad_v, grad_w, gt_sdf)
    res = _run(in_maps, trace=False)
    return _finalize(res.results)
